# revision 1
# baseline (speedup 1.0000x reference)
"""Trainium2 Bass kernel for the dual-softmax cross-attention module.

Sharding: 8 cores = batch (4) x head-half (2).  Core c handles batch c//2 and
heads 4*(c%2) .. 4*(c%2)+4.  Each core computes Q/K/V projections for its
head-group, the 2048x2048 score matrix per head, one shared E = exp(s/8)
(both softmaxes are shift-invariant; scores are O(1) so no max subtraction),
contexts for both streams, exchanges context halves with its pair core via a
2-core AllGather, and produces a disjoint 256-channel slice of both outputs.

All matmuls run in bf16 (fp32 PSUM accumulation); residual + output stay fp32.
"""

import sys

for _p in ("/opt/trn_rl_repo", "/opt/pypackages"):
    if _p not in sys.path:
        sys.path.insert(0, _p)

import numpy as np
import ml_dtypes

import concourse.bass as bass
import concourse.tile as tile
from concourse import bacc, mybir
from concourse.bass_utils import run_bass_kernel_spmd

F32 = mybir.dt.float32
BF16 = mybir.dt.bfloat16
AF = mybir.ActivationFunctionType
AX = mybir.AxisListType

N_CORES = 8
B = 4          # batch
C = 512        # channels
N = 2048       # tokens (8*16*16)
H = 8          # heads
DH = 64        # head dim
HL = 4         # heads per core
CL = 256       # channels per core (head-group)
NT = N // 128  # 16 token tiles
CT = C // 128  # 4 channel tiles

_BF = ml_dtypes.bfloat16


def _build():
    nc = bacc.Bacc("TRN2", target_bir_lowering=False, debug=False,
                   num_devices=N_CORES)

    def din(name, shape, dt=BF16):
        return nc.dram_tensor(name, shape, dt, kind="ExternalInput").ap()

    x1b = din("x1b", [CT, 128, N])          # x1[b] channel-major, bf16
    x2b = din("x2b", [CT, 128, N])
    wq = din("wq", [128, CT, CL])           # column slice of Wq, pre-permuted
    wk = din("wk", [128, CT, CL])
    wv1 = din("wv1", [128, CT, CL])
    wv2 = din("wv2", [128, CT, CL])
    wo1 = din("wo1", [128, CT, CL])         # Wo columns for my output rows
    wo2 = din("wo2", [128, CT, CL])
    bq = din("bq", [128, 2, 1], F32)        # bias slices per M-tile
    bk = din("bk", [128, 2, 1], F32)
    bv1 = din("bv1", [1, CL])
    bv2 = din("bv2", [1, CL])
    x1r = din("x1r", [2, 128, N], F32)      # x1[b] residual slice + bo1
    x2r = din("x2r", [2, 128, N], F32)

    o1 = nc.dram_tensor("o1", [2, 128, N], F32, kind="ExternalOutput").ap()
    o2 = nc.dram_tensor("o2", [2, 128, N], F32, kind="ExternalOutput").ap()

    with tile.TileContext(nc) as tc:
        _emit(nc, tc, locals())
    nc.compile()
    return nc


def _emit(nc, tc, t):
    x1b, x2b = t["x1b"], t["x2b"]
    wq, wk, wv1, wv2 = t["wq"], t["wk"], t["wv1"], t["wv2"]
    wo1, wo2 = t["wo1"], t["wo2"]
    bq, bk, bv1, bv2 = t["bq"], t["bk"], t["bv1"], t["bv2"]
    x1r, x2r, o1, o2 = t["x1r"], t["x2r"], t["o1"], t["o2"]

    from contextlib import ExitStack
    ctx = ExitStack()
    with ctx:
        persist = ctx.enter_context(tc.tile_pool(name="persist", bufs=1))
        small = ctx.enter_context(tc.tile_pool(name="small", bufs=8))
        vp_pool = ctx.enter_context(tc.tile_pool(name="vp", bufs=4))
        dram = ctx.enter_context(tc.tile_pool(name="dram", bufs=2, space="DRAM"))

        # ---- persistent SBUF tensors (packed to dodge 4KB tile padding) ----
        w_all = persist.tile([128, 6, CT, CL], BF16, tag="wall")
        wq_s, wk_s, wv1_s, wv2_s, wo1_s, wo2_s = (w_all[:, i, :, :]
                                                  for i in range(6))
        bqk_s = persist.tile([128, 4, 1], F32, tag="bqk")
        bq_s, bk_s = bqk_s[:, 0:2, :], bqk_s[:, 2:4, :]
        ones_full = persist.tile([128, N], BF16, tag="ones", name="ones_full")
        ones_s = ones_full[0:1, :]
        misc_s = persist.tile([128, 640], BF16, tag="misc")
        bv1_s = misc_s[0:1, 0:CL]
        bv2_s = misc_s[0:1, CL:2 * CL]
        onec_s = misc_s[:, 512:513]
        qt_s = persist.tile([128, 2, N], BF16, tag="qt")    # Q^T  (chan-major)
        kt_s = persist.tile([128, 2, N], BF16, tag="kt")    # K^T
        v1tok = persist.tile([128, NT, CL], BF16, tag="v1tok")  # token-major V1
        v2tok = persist.tile([128, NT, CL], BF16, tag="v2tok")
        cm = {}  # gathered ctx^T tiles; pool opened once xb tiles retire

        for i, src in enumerate((wq, wk, wv1, wv2, wo1, wo2)):
            nc.sync.dma_start(w_all[:, i, :, :], src[:, :, :])
        nc.sync.dma_start(bq_s[:, :, :], bq[:, :, :])
        nc.sync.dma_start(bk_s[:, :, :], bk[:, :, :])
        nc.sync.dma_start(bv1_s[:, :], bv1[:, :])
        nc.sync.dma_start(bv2_s[:, :], bv2[:, :])
        nc.vector.memset(ones_s[:, :], 1.0)
        nc.vector.memset(onec_s[:, :], 1.0)

        # ---- P1: x loads + Q/K projections (V projections are interleaved
        # into head 0's qtile loop, using the then-idle ctx1 psum slot) ----
        # SBUF pool stacking: p2's SBUF pools open first, then xb (which is
        # released after head 0 so the gathered-context buffers reuse it).
        p2 = ExitStack()
        eslab = p2.enter_context(tc.tile_pool(name="eslab", bufs=6))
        et_pool = p2.enter_context(tc.tile_pool(name="et", bufs=1))
        gsrc_pool = p2.enter_context(tc.tile_pool(name="gsrc", bufs=2))
        csrow_pool = p2.enter_context(tc.tile_pool(name="csrow", bufs=1))
        p1 = ExitStack()
        pj_ps = p1.enter_context(tc.tile_pool(name="pj_ps", bufs=2, space="PSUM"))
        xb_stack = ExitStack()
        xb_pool = xb_stack.enter_context(tc.tile_pool(name="xb", bufs=8))
        xts = {}
        for xi, xb_dram in enumerate((x1b, x2b)):
            xts[xi] = [xb_pool.tile([128, N], BF16, tag="xb", name=f"xt{xi}_{i}")
                       for i in range(CT)]
            for ti in range(CT):
                nc.sync.dma_start(xts[xi][ti][:, :], xb_dram[ti, :, :])
        # chan-major Q/K:  out[cl, n] = sum_cin W[cin, cl] * x[cin, n]
        for xi, w_qk, b_qk, qk_dst in ((0, wq_s, bq_s, qt_s),
                                       (1, wk_s, bk_s, kt_s)):
            for m in range(2):
                for half in range(2):
                    ps = pj_ps.tile([128, 1024], F32, tag="pj")
                    for ch in range(2):
                        off = half * 1024 + ch * 512
                        for ti in range(CT):
                            nc.tensor.matmul(
                                ps[:, ch * 512:(ch + 1) * 512],
                                w_qk[:, ti, m * 128:(m + 1) * 128],
                                xts[xi][ti][:, off:off + 512],
                                start=(ti == 0), stop=(ti == CT - 1))
                    nc.scalar.activation(
                        qk_dst[:, m, half * 1024:(half + 1) * 1024], ps[:, :],
                        AF.Identity, bias=b_qk[:, m, :])
        p1.close()

        def emit_v_proj(xi, w_v, b_v, v_dst, nt, vps_pool):
            # token-major V:  out[n, cl] = sum_cin x[cin, n] * W[cin, cl] + bv
            ps = vps_pool.tile([128, 512], F32, tag="c1", name=f"vps{xi}_{nt}")
            for ti in range(CT):
                nc.tensor.matmul(
                    ps[:, 0:CL], xts[xi][ti][:, nt * 128:(nt + 1) * 128],
                    w_v[:, ti, :], start=(ti == 0), stop=False)
            nc.tensor.matmul(ps[:, 0:CL], ones_s[:, nt * 128:(nt + 1) * 128],
                             b_v[:, :], start=False, stop=True)
            nc.vector.tensor_copy(v_dst[:, nt, :], ps[:, 0:CL])

        # ---- P2: per-head attention, software-pipelined across heads ----
        # Per head hl, the qtile loop streams: scores -> exp(+rowsum chunk
        # accum) -> ctx2 (ones-augmented lhsT, so PSUM row 64 accumulates
        # colsum for free; emission lags one qtile) -> E^T transpose (lags 4).
        # Interleaved into head hl's loop is head hl-1's epilogue: ctx2 evac,
        # colsum row->column (16 K=1 matmuls), ctx1 spread ch-major over 8
        # qtiles (1-bank psum tiles; et stripes release per-ch), AllGather.
        sc_ps = p2.enter_context(tc.tile_pool(name="sc_ps", bufs=2, space="PSUM"))
        c2_ps = p2.enter_context(tc.tile_pool(name="c2_ps", bufs=1, space="PSUM"))
        c1_ps = p2.enter_context(tc.tile_pool(name="c1_ps", bufs=2, space="PSUM"))

        st = {}  # per-head pipeline state

        def head_slices(hl):
            g, poff = hl // 2, 64 * (hl % 2)
            return (qt_s[poff:poff + 64, g, :], kt_s[poff:poff + 64, g, :], poff)

        def emit_scores_exp(hl, qt):
            q_l, k_l, _ = head_slices(hl)
            s = st[hl]
            es = eslab.tile([128, N], BF16, tag="es", name=f"es{hl}_{qt}")
            sq = small.tile([128, 24], F32, tag="sq", bufs=4,
                            name=f"sq{hl}_{qt}")
            rs_p, rs, rr = sq[:, 0:3], sq[:, 4:5], sq[:, 5:6]
            for u in range(4):
                ps = sc_ps.tile([128, 512], F32, tag="sc", name=f"sps{u}")
                nc.tensor.matmul(ps[:, :], q_l[:, qt * 128:(qt + 1) * 128],
                                 k_l[:, u * 512:(u + 1) * 512],
                                 start=True, stop=True)
                # rowsum split: chunks 0-1 use the ACT fused accumulator,
                # chunks 2-3 are reduced on DVE in one op below
                nc.scalar.activation(es[:, u * 512:(u + 1) * 512], ps[:, :],
                                     AF.Exp, scale=0.125,
                                     accum_out=(rs_p[:, u:u + 1]
                                                if u < 2 else None))
            nc.vector.reduce_sum(out=rs_p[:, 2:3], in_=es[:, 1024:2048],
                                 axis=AX.X)
            nc.vector.reduce_sum(out=rs[:, :], in_=rs_p[:, :], axis=AX.X)
            nc.vector.reciprocal(rr[:, :], rs[:, :])
            if qt % 4 == 0:
                s["v2pk"] = vp_pool.tile([128, 4, DH + 1], BF16, tag="v2p",
                                         bufs=2, name=f"v2pk{hl}_{qt}")
            v2p = s["v2pk"][:, qt % 4, :]
            nc.vector.tensor_scalar_mul(
                v2p[:, 0:DH], v2tok[:, qt, hl * DH:(hl + 1) * DH], rr[:, :])
            nc.vector.memset(v2p[:, DH:DH + 1], 1.0)
            s["es"][qt] = es
            s["v2p"][qt] = v2p

        def emit_ctx2(hl, qt):
            s = st[hl]
            for ch in range(4):
                nc.tensor.matmul(
                    s["cps2"][0:DH + 1, ch * 512:(ch + 1) * 512],
                    s["v2p"][qt][:, :], s["es"][qt][:, ch * 512:(ch + 1) * 512],
                    start=(qt == 0), stop=(qt == NT - 1))

        def emit_transpose(hl, qt):
            s = st[hl]
            if s["et"] is None:
                s["et"] = et_pool.tile([128, NT, N], BF16, tag="et",
                                       name=f"et{hl}")
            nc.sync.dma_start(
                s["et"][:, qt, :].rearrange("p (a b) -> p a b", b=128),
                s["es"][qt][:, :], transpose=True)

        def emit_epilogue_a(hl):
            # copy colsum row out of psum FIRST (it gates the next head's
            # colsum matmuls / ctx1 chain on the PE), then evac ctx2
            s = st[hl]
            csrow = csrow_pool.tile([65, N], BF16, tag="csr", name=f"csr{hl}")
            s["csrow"] = csrow
            nc.vector.tensor_copy(csrow[64:65, :], s["cps2"][64:65, :])
            gs = gsrc_pool.tile([128, N], BF16, tag="gs", name=f"gs{hl}")
            s["gs"] = gs
            nc.vector.tensor_copy(gs[0:64, :], s["cps2"][0:64, :])

        def emit_epilogue_b(hl):
            # colsum row -> column via 16 K=1 matmuls, recip, scale v1
            s = st[hl]
            cs_ps = sc_ps.tile([128, 512], F32, tag="sc", name=f"cs_ps{hl}")
            for kt in range(NT):
                nc.tensor.matmul(cs_ps[:, kt:kt + 1],
                                 s["csrow"][64:65, kt * 128:(kt + 1) * 128],
                                 onec_s[64:65, :], start=True, stop=True)
            cr_t = small.tile([128, NT], F32, tag="cr", bufs=2, name=f"cr{hl}")
            nc.vector.reciprocal(cr_t[:, :], cs_ps[:, 0:NT])
            v1pk = vp_pool.tile([128, NT, DH], BF16, tag="v1p", bufs=2,
                                name=f"v1pk{hl}")
            for kt in range(NT):
                nc.vector.tensor_scalar_mul(
                    v1pk[:, kt, :], v1tok[:, kt, hl * DH:(hl + 1) * DH],
                    cr_t[:, kt:kt + 1])
                s["v1p"][kt] = v1pk[:, kt, :]

        def emit_ctx1_step(hl, step):
            # step 0..11: ch = step//3, kt third = step%3 (6/5/5 kts)
            s = st[hl]
            ch, third = step // 3, step % 3
            kt_lo, kt_hi = (0, 6) if third == 0 else (
                (6, 11) if third == 1 else (11, 16))
            if third == 0:
                s["c1"][ch] = c1_ps.tile([128, 512], F32, tag="c1",
                                         name=f"c1_{hl}_{ch}")
            for kt in range(kt_lo, kt_hi):
                nc.tensor.matmul(
                    s["c1"][ch][64:128, :], s["v1p"][kt][:, :],
                    s["et"][:, 4 * ch:4 * (ch + 1), kt * 128:(kt + 1) * 128],
                    start=(kt == 0), stop=(kt == NT - 1))
            if third == 2:
                nc.vector.tensor_copy(
                    s["gs"][64:128, ch * 512:(ch + 1) * 512],
                    s["c1"][ch][64:128, :])

        def emit_gather(hl, half=None):
            # half=None: gather both ctx halves; 0: ctx2 rows only; 1: ctx1
            s = st[hl]
            _, _, poff = head_slices(hl)
            rows = slice(0, 128) if half is None else (
                slice(0, 64) if half == 0 else slice(64, 128))
            nr = rows.stop - rows.start
            sfx = f"{hl}_{half}"
            gin = dram.tile([nr, N], BF16, tag="gin", name=f"gin{sfx}")
            gout = dram.tile([2, nr, N], BF16, tag="gout", bufs=4,
                             name=f"gout{sfx}")
            nc.gpsimd.dma_start(gin[:, :], s["gs"][rows, :])
            nc.gpsimd.collective_compute(
                "AllGather", mybir.AluOpType.bypass,
                replica_groups=[[0, 1], [2, 3], [4, 5], [6, 7]],
                ins=[gin.opt()], outs=[gout.opt()])
            for r in range(2):
                tt = 2 * r + hl // 2
                if half in (None, 0):
                    nc.sync.dma_start(cm["2"][poff:poff + 64, tt, :],
                                      gout[r, 0:64, :])
                if half in (None, 1):
                    ro = 64 if half is None else 0
                    nc.sync.dma_start(cm["1"][poff:poff + 64, tt, :],
                                      gout[r, ro:ro + 64, :])

        def emit_head_qt(hl, qt):
            # one qtile of head hl + interleaved epilogue work of head hl-1
            # (or, for head 0, the V projections)
            if hl == 0:
                emit_v_proj(1, wv2_s, bv2_s, v2tok, qt, c1_ps)
            emit_scores_exp(hl, qt)
            if hl == 0:
                emit_v_proj(0, wv1_s, bv1_s, v1tok, qt, c1_ps)
            else:
                if qt == 1:
                    emit_epilogue_b(hl - 1)
                elif 2 <= qt <= 13:
                    emit_ctx1_step(hl - 1, qt - 2)
                elif qt == 14:
                    emit_gather(hl - 1)
            if qt > 0:
                emit_ctx2(hl, qt - 1)
            if qt >= 4:
                emit_transpose(hl, qt - 4)

        for hl in range(HL):
            st[hl] = {"es": {}, "v2p": {}, "v1p": {}, "c1": {}, "et": None,
                      "cps2": c2_ps.tile([128, N], F32, tag="c2",
                                         name=f"cps2_{hl}")}
            for qt in range(NT):
                emit_head_qt(hl, qt)
            emit_ctx2(hl, NT - 1)
            emit_epilogue_a(hl)
            for qt in range(NT - 4, NT):
                emit_transpose(hl, qt)
            if hl == 0:
                # x tiles retire with head 0's V projections; reuse their
                # SBUF for the gathered-context buffers
                xb_stack.close()
                cm_pool = p2.enter_context(tc.tile_pool(name="cm", bufs=1))
                cm["1"] = cm_pool.tile([128, CT, N], BF16, tag="ctxm1",
                                       name="ctxm1")
                cm["2"] = cm_pool.tile([128, CT, N], BF16, tag="ctxm2",
                                       name="ctxm2")
        # epilogue of the last head: ship the ctx2 half while ctx1 computes
        emit_gather(HL - 1, half=0)
        emit_epilogue_b(HL - 1)
        for step in range(12):
            emit_ctx1_step(HL - 1, step)
        emit_gather(HL - 1, half=1)

        p2.close()

        # ---- P3: output projections + residual ----
        p3 = ExitStack()
        o_ps = p3.enter_context(tc.tile_pool(name="o_ps", bufs=2, space="PSUM"))
        xr_pool = p3.enter_context(tc.tile_pool(name="xr", bufs=2))
        out_pool = p3.enter_context(tc.tile_pool(name="outp", bufs=2))
        for w_s, cmt, xr, oo in ((wo2_s, cm["2"], x2r, o2),
                                 (wo1_s, cm["1"], x1r, o1)):
            for m in range(2):
                xr_t = xr_pool.tile([128, N], F32, tag="xr")
                nc.sync.dma_start(xr_t[:, :], xr[m, :, :])
                ps = o_ps.tile([128, N], F32, tag="o")
                # tiles 0,2 hold heads 0-5 (ready after gather(1)); tiles
                # 1,3 need the last gather -- accumulate those last
                for tis in ((0, 2), (1, 3)):
                    for ch in range(4):
                        for ti in tis:
                            nc.tensor.matmul(
                                ps[:, ch * 512:(ch + 1) * 512],
                                w_s[:, ti, m * 128:(m + 1) * 128],
                                cmt[:, ti, ch * 512:(ch + 1) * 512],
                                start=(ti == 0), stop=(ti == 3))
                ot = out_pool.tile([128, N], F32, tag="ot")
                nc.vector.tensor_add(ot[:, :], ps[:, :], xr_t[:, :])
                nc.sync.dma_start(oo[m, :, :], ot[:, :])
        p3.close()


_NC_CACHE = None


def _get_nc():
    global _NC_CACHE
    if _NC_CACHE is None:
        _NC_CACHE = _build()
    return _NC_CACHE


def _in_maps(x1, x2, Wq, bq, Wk, bk, Wv1, bv1, Wv2, bv2, Wo1, bo1, Wo2, bo2):
    x1f = np.asarray(x1, np.float32).reshape(B, C, N)
    x2f = np.asarray(x2, np.float32).reshape(B, C, N)
    in_maps = []
    for c in range(N_CORES):
        b, hq = c // 2, c % 2
        sl = slice(CL * hq, CL * hq + CL)
        def wslice(W):
            return np.ascontiguousarray(
                np.asarray(W, np.float32)[:, sl].reshape(CT, 128, CL)
                .transpose(1, 0, 2)).astype(_BF)

        m = {
            "x1b": x1f[b].reshape(CT, 128, N).astype(_BF),
            "x2b": x2f[b].reshape(CT, 128, N).astype(_BF),
            "wq": wslice(Wq), "wk": wslice(Wk),
            "wv1": wslice(Wv1), "wv2": wslice(Wv2),
            "wo1": wslice(Wo1), "wo2": wslice(Wo2),
            "bq": np.ascontiguousarray(
                np.asarray(bq, np.float32)[sl].reshape(2, 128).T).reshape(128, 2, 1),
            "bk": np.ascontiguousarray(
                np.asarray(bk, np.float32)[sl].reshape(2, 128).T).reshape(128, 2, 1),
            "bv1": np.asarray(bv1, np.float32)[sl].reshape(1, CL).astype(_BF),
            "bv2": np.asarray(bv2, np.float32)[sl].reshape(1, CL).astype(_BF),
            "x1r": (x1f[b, sl, :] + np.asarray(bo1, np.float32)[sl, None]
                    ).reshape(2, 128, N),
            "x2r": (x2f[b, sl, :] + np.asarray(bo2, np.float32)[sl, None]
                    ).reshape(2, 128, N),
        }
        in_maps.append(m)
    return in_maps


def _unshard(res):
    o1 = np.empty((B, C, N), np.float32)
    o2 = np.empty((B, C, N), np.float32)
    for c in range(N_CORES):
        b, hq = c // 2, c % 2
        sl = slice(CL * hq, CL * hq + CL)
        o1[b, sl, :] = res[c]["o1"].reshape(CL, N)
        o2[b, sl, :] = res[c]["o2"].reshape(CL, N)
    shape = (B, C, 8, 16, 16)
    return o1.reshape(shape), o2.reshape(shape)


def kernel(**inputs):
    in_maps = _in_maps(**inputs)
    nc = _get_nc()
    res = run_bass_kernel_spmd(nc, in_maps, list(range(N_CORES))).results
    return _unshard(res)



# revision 20
# speedup vs baseline: 1.2418x; 1.2418x over previous
"""Trainium2 Bass kernel for the dual-softmax cross-attention module.

Sharding: 8 cores = batch (4) x head-half (2).  Core c handles batch c//2 and
heads 4*(c%2) .. 4*(c%2)+4, producing a disjoint 256-channel slice of both
outputs (context halves exchanged with the pair core via a 2-core AllGather).

Per head: scores run in bf16 (K=64); one big exp ACTIVATE per q-tile writes
E straight to fp8 with the row-sum accumulated on the ACT side.  Both context
matmuls and the output projections run as fp8 DoubleRow (two 128-deep
contraction blocks per instruction).  E^T for ctx1 is produced by DMA-xbar
transposes of the fp8 E viewed as uint16 pairs; the resulting parity
interleave is absorbed by stride-2 access patterns and a pre-permuted
token-major V1.  v1/v2 are pre-scaled by 1024/colsum resp. 1024/rowsum, Wo by
64; the output projection evac fuses the 2^-16 unscale with the residual add.
"""

import sys

for _p in ("/opt/trn_rl_repo", "/opt/pypackages"):
    if _p not in sys.path:
        sys.path.insert(0, _p)

import numpy as np
import ml_dtypes

import concourse.bass as bass
import concourse.tile as tile
from concourse import bacc, mybir
from concourse.bass_utils import run_bass_kernel_spmd

F32 = mybir.dt.float32
BF16 = mybir.dt.bfloat16
FP8 = mybir.dt.float8e4
U16 = mybir.dt.uint16
AF = mybir.ActivationFunctionType
ALU = mybir.AluOpType
DR = mybir.MatmulPerfMode.DoubleRow

N_CORES = 8
B = 4          # batch
C = 512        # channels
N = 2048       # tokens (8*16*16)
H = 8          # heads
DH = 64        # head dim
HL = 4         # heads per core
CL = 256       # channels per core (head-group)
NT = N // 128  # 16 token tiles
CT = C // 128  # 4 channel tiles
ES_RING = 22   # q-tile ring slots for E (fp8)
ET_RING = 26   # ring slots for E^T
VS = 1024.0    # v1/v2 scale (keeps fp8 operands in range)
WOS = 64.0     # Wo scale
OUS = 1.0 / (VS * WOS)  # output unscale

_BF = ml_dtypes.bfloat16
_F8 = ml_dtypes.float8_e4m3


def _build():
    nc = bacc.Bacc("TRN2", target_bir_lowering=False, debug=False,
                   num_devices=N_CORES)

    def din(name, shape, dt=BF16):
        return nc.dram_tensor(name, shape, dt, kind="ExternalInput").ap()

    x1b = din("x1b", [CT, 128, N])          # x1[b] channel-major, bf16
    x2b = din("x2b", [CT, 128, N])
    wq = din("wq", [128, CT, CL])           # column slice of Wq, pre-permuted
    wk = din("wk", [128, CT, CL])
    wv1 = din("wv1", [128, CT, CL])
    wv2 = din("wv2", [128, CT, CL])
    wo1 = din("wo1", [128, CT, CL], FP8)    # Wo columns for my rows, x64 fp8
    wo2 = din("wo2", [128, CT, CL], FP8)
    bq = din("bq", [128, 2, 1], F32)        # bias slices per M-tile
    bk = din("bk", [128, 2, 1], F32)
    bv1 = din("bv1", [1, CL])
    bv2 = din("bv2", [1, CL])
    x1r = din("x1r", [2, 128, N], BF16)     # x1[b] residual slice + bo1
    x2r = din("x2r", [2, 128, N], BF16)

    o1 = nc.dram_tensor("o1", [2, 128, N], F32, kind="ExternalOutput").ap()
    o2 = nc.dram_tensor("o2", [2, 128, N], F32, kind="ExternalOutput").ap()

    with tile.TileContext(nc) as tc:
        _emit(nc, tc, locals())
    nc.compile()
    return nc


def _emit(nc, tc, t):
    x1b, x2b = t["x1b"], t["x2b"]
    wq, wk, wv1, wv2 = t["wq"], t["wk"], t["wv1"], t["wv2"]
    wo1, wo2 = t["wo1"], t["wo2"]
    bq, bk, bv1, bv2 = t["bq"], t["bk"], t["bv1"], t["bv2"]
    x1r, x2r, o1, o2 = t["x1r"], t["x2r"], t["o1"], t["o2"]

    from contextlib import ExitStack
    ctx = ExitStack()
    with ctx:
        persist = ctx.enter_context(tc.tile_pool(name="persist", bufs=1))
        dram = ctx.enter_context(tc.tile_pool(name="dram", bufs=2, space="DRAM"))

        # ---- persistent SBUF tensors ----
        w_all = persist.tile([128, 4, CT, CL], BF16, tag="wall")
        wq_s, wk_s, wv1_s, wv2_s = (w_all[:, i, :, :] for i in range(4))
        wo_all = persist.tile([128, 2, CT, CL], FP8, tag="woall")
        wo1_s, wo2_s = wo_all[:, 0, :, :], wo_all[:, 1, :, :]
        bqk_s = persist.tile([128, 4, 1], F32, tag="bqk")
        bq_s, bk_s = bqk_s[:, 0:2, :], bqk_s[:, 2:4, :]
        misc = persist.tile([128, 640], BF16, tag="misc")
        bv1_s = misc[0:1, 0:CL]
        bv2_s = misc[0:1, CL:2 * CL]
        ones_s = misc[0:1, 512:640]         # ones row for V bias matmul
        qt_s = persist.tile([128, 2, N], BF16, tag="qt")    # Q^T (chan-major)
        kt_s = persist.tile([128, 2, N], BF16, tag="kt")
        v1tok = persist.tile([128, NT, CL], BF16, tag="v1tok")  # PERMUTED ord
        v2tok = persist.tile([128, NT, CL], BF16, tag="v2tok")  # natural ord
        v2p = persist.tile([128, 2, NT, 80], FP8, tag="v2p")    # 64 + 2 ones
        # (pair-dim stride must be a multiple of 16 for dual-fp8 ldweights)
        v1q = persist.tile([128, 2, NT, DH], FP8, tag="v1q")
        rsml = persist.tile([128, 2, 3 * NT], F32, tag="rsml")
        rs_t = rsml[:, :, 0:NT]             # rowsum accum (per qt)
        rss_t = rsml[:, :, NT:2 * NT]       # rowsum * 2^-10
        rr_t = rsml[:, :, 2 * NT:3 * NT]    # VS / rowsum
        csrow = persist.tile([1, N], BF16, tag="csrow")     # colsum row *2^-10
        csc = persist.tile([128, 2 * NT], F32, tag="csc")
        cscol = csc[:, 0:NT]                # colsum col (permuted order)
        crq = csc[:, NT:2 * NT]             # VS / colsum
        cm = {"1": persist.tile([128, CT, N], FP8, tag="cm1", name="cm1"),
              "2": persist.tile([128, CT, N], FP8, tag="cm2", name="cm2")}
        gs_t = persist.tile([128, 2, N], FP8, tag="gs")     # ctx2 staging
        gs1_t = persist.tile([64, 2, N], FP8, tag="gs1")    # ctx1 staging
        xres_box = {}  # [m, stream] residual tile; pool reuses xb's space

        # ---- E / E^T rings (sub-tile AP dependency tracking) ----
        es8 = persist.tile([128, ES_RING, N], FP8, tag="es8")
        esT = persist.tile([128, 8, ET_RING, 256], FP8, tag="esT")
        es8_u16 = es8.bitcast(U16)          # [128, ES_RING, 1024]
        esT_u16 = esT.bitcast(U16)          # [128, 8, ET_RING, 128]

        # ---- input DMA (gpsimd software DGE; sync is reserved for xbar) ----
        for i, src in enumerate((wq, wk, wv1, wv2)):
            nc.gpsimd.dma_start(w_all[:, i, :, :], src[:, :, :])
        nc.gpsimd.dma_start(wo_all[:, 0, :, :], wo1[:, :, :])
        nc.gpsimd.dma_start(wo_all[:, 1, :, :], wo2[:, :, :])
        nc.gpsimd.dma_start(bq_s[:, :, :], bq[:, :, :])
        nc.gpsimd.dma_start(bk_s[:, :, :], bk[:, :, :])
        nc.gpsimd.dma_start(bv1_s[:, :], bv1[:, :])
        nc.gpsimd.dma_start(bv2_s[:, :], bv2[:, :])
        nc.vector.memset(ones_s[:, :], 1.0)
        nc.vector.memset(v2p[:, :, :, DH:DH + 2], 1.0)

        # ---- P1: x loads + Q/K projections ----
        p1 = ExitStack()
        pj_ps = p1.enter_context(tc.tile_pool(name="pj_ps", bufs=2, space="PSUM"))
        xb_stack = ExitStack()
        xb_pool = xb_stack.enter_context(tc.tile_pool(name="xb", bufs=8))
        xts = {}
        for xi, xb_dram in enumerate((x1b, x2b)):
            xts[xi] = [xb_pool.tile([128, N], BF16, tag="xb", name=f"xt{xi}_{i}")
                       for i in range(CT)]
            for ti in range(CT):
                nc.gpsimd.dma_start(xts[xi][ti][:, :], xb_dram[ti, :, :])
        # chan-major Q/K: out[cl, n] = sum_cin W[cin, cl] * x[cin, n] + b
        # m=0 (heads 0-1) first so head 0's scores can start early.
        for m in range(2):
            for xi, w_qk, b_qk, qk_dst in ((0, wq_s, bq_s, qt_s),
                                           (1, wk_s, bk_s, kt_s)):
                for half in range(2):
                    ps = pj_ps.tile([128, 1024], F32, tag="pj")
                    for ch in range(2):
                        off = half * 1024 + ch * 512
                        for ti in range(CT):
                            nc.tensor.matmul(
                                ps[:, ch * 512:(ch + 1) * 512],
                                w_qk[:, ti, m * 128:(m + 1) * 128],
                                xts[xi][ti][:, off:off + 512],
                                start=(ti == 0), stop=(ti == CT - 1))
                    nc.vector.tensor_scalar_add(
                        qk_dst[:, m, half * 1024:(half + 1) * 1024], ps[:, :],
                        b_qk[:, m, :])
        p1.close()

        # ---- head-phase PSUM pools: scores 4 banks, ctx2 2, ctx1 2 ----
        p2 = ExitStack()
        sc_ps = p2.enter_context(tc.tile_pool(name="sc_ps", bufs=1, space="PSUM"))
        c2_ps = p2.enter_context(tc.tile_pool(name="c2_ps", bufs=1, space="PSUM"))
        c1_ps = p2.enter_context(tc.tile_pool(name="c1_ps", bufs=2, space="PSUM"))

        st = {}  # per-head state

        def head_slices(hl):
            g, poff = hl // 2, 64 * (hl % 2)
            return (qt_s[poff:poff + 64, g, :], kt_s[poff:poff + 64, g, :], poff)

        def es_slot(hl, qt):
            return (16 * hl + qt) % ES_RING

        def et_slot(hl, qt):
            return (16 * hl + qt) % ET_RING

        def emit_v_proj(xi, w_v, b_v, v_dst, sl, permute):
            # token-major V: out[n, cl] = sum_cin x[cin, n] * W[cin, cl] + bv
            # permute: stationary picks tokens 256*(sl//2) + (sl%2) :: 2
            ps = c1_ps.tile([128, 512], F32, tag="c1", name=f"vps{xi}_{sl}")
            for ti in range(CT):
                xt = xts[xi][ti]
                if permute:
                    a0 = 128 * (sl // 2)
                    j = sl % 2
                    src = (xt[:, :].rearrange("p (a s) -> p a s", s=2)
                           [:, a0:a0 + 128, j:j + 1])
                else:
                    src = xt[:, sl * 128:(sl + 1) * 128]
                nc.tensor.matmul(ps[:, 0:CL], src, w_v[:, ti, :],
                                 start=(ti == 0), stop=False)
            nc.tensor.matmul(ps[:, 0:CL], ones_s[:, 0:128], b_v[:, :],
                             start=False, stop=True)
            nc.vector.tensor_copy(v_dst[:, sl, :], ps[:, 0:CL])

        def emit_scores(hl, qt):
            q_l, k_l, _ = head_slices(hl)
            ps = sc_ps.tile([128, N], F32, tag="sc", name=f"sc{hl}_{qt}")
            st[hl]["sc"] = ps
            for u in range(4):
                nc.tensor.matmul(ps[:, u * 512:(u + 1) * 512],
                                 q_l[:, qt * 128:(qt + 1) * 128],
                                 k_l[:, u * 512:(u + 1) * 512],
                                 start=True, stop=True)

        def emit_exp(hl, qt):
            s = es_slot(hl, qt)
            pp = hl % 2
            nc.scalar.activation(es8[:, s, :], st[hl]["sc"][:, :], AF.Exp,
                                 scale=0.125, accum_out=rs_t[:, pp, qt:qt + 1])

        def emit_transpose(hl, qt):
            s, ts = es_slot(hl, qt), et_slot(hl, qt)
            nc.sync.dma_start(esT_u16[:, :, ts, :], es8_u16[:, s, :],
                              transpose=True)

        def emit_v2p_group(hl, g):
            # rr = VS / rowsum for qt in [4g, 4g+4); scale v2 -> fp8
            pp = hl % 2
            sl4 = slice(4 * g, 4 * g + 4)
            nc.vector.tensor_scalar_mul(rss_t[:, pp, sl4], rs_t[:, pp, sl4],
                                        1.0 / VS)
            nc.vector.reciprocal(rr_t[:, pp, sl4], rss_t[:, pp, sl4])
            for qt in range(4 * g, 4 * g + 4):
                nc.vector.tensor_scalar_mul(
                    v2p[:, pp, qt, 0:DH], v2tok[:, qt, hl * DH:(hl + 1) * DH],
                    rr_t[:, pp, qt:qt + 1])

        def emit_ctx2_pairs(hp, half, pairs):
            # ctx2 chan-major [65, 1024-half], fp8 DoubleRow over qt pairs
            pp = hp % 2
            s = st[hp]
            for p in pairs:
                if p == 0:
                    s["c2"] = c2_ps.tile([128, 1024], F32, tag="c2",
                                         name=f"c2_{hp}_{half}")
                sl0 = es_slot(hp, 2 * p)
                for ch in range(2):
                    ksl = slice(half * 1024 + ch * 512,
                                half * 1024 + (ch + 1) * 512)
                    nc.tensor.matmul(
                        s["c2"][0:DH + 2, ch * 512:(ch + 1) * 512],
                        v2p[:, pp, 2 * p:2 * p + 2, 0:DH + 2],
                        es8[:, sl0:sl0 + 2, ksl],
                        start=(p == 0), stop=(p == 7), perf_mode=DR)

        def emit_ctx2_evac(hp, half):
            pp = hp % 2
            s = st[hp]
            nc.vector.tensor_copy(gs_t[0:DH, pp, half * 1024:(half + 1) * 1024],
                                  s["c2"][0:DH, :])
            # colsum row, written directly in the permuted (u, s=2*ub+j)
            # order v1tok uses: csrow[u*16 + 2*ub + j] = colsum[256*ub+2u+j]
            dst = (csrow[0:1, :]
                   .rearrange("o (u ub j) -> o ub u j", u=128, ub=8, j=2)
                   [:, 4 * half:4 * half + 4, :, :])
            nc.vector.tensor_scalar_mul(dst, s["c2"][DH:DH + 1, :], 1.0 / VS)

        def emit_cs_col(hp):
            # csrow is already in (u, s) order; spread over 128 partitions
            nc.gpsimd.dma_start(cscol[:, :], csrow[0:1, :])

        def emit_cs_col2(hp):
            nc.vector.reciprocal(crq[:, :], cscol[:, :])
            pp = hp % 2
            for sl in range(NT):
                nc.vector.tensor_scalar_mul(
                    v1q[:, pp, sl, :], v1tok[:, sl, hp * DH:(hp + 1) * DH],
                    crq[:, sl:sl + 1])

        def emit_ctx1_chunk(hp, j):
            # ctx1 chan-major [64 d, 256 q]: lhsT = v1q pair (stationary),
            # rhs = E^T pairs with the parity interleave as the DoubleRow dim
            pp = hp % 2
            ps = c1_ps.tile([128, 512], F32, tag="c1", name=f"c1_{hp}_{j}")
            ts0 = et_slot(hp, 2 * j)
            rhs = (esT[:, :, ts0:ts0 + 2, :]
                   .rearrange("p ub t (q j) -> p ub j t q", j=2))
            for ub in range(8):
                nc.tensor.matmul(
                    ps[0:64, 0:256],
                    v1q[:, pp, 2 * ub:2 * ub + 2, :],
                    rhs[:, ub, :, :, :],
                    start=(ub == 0), stop=(ub == 7), perf_mode=DR)
            nc.vector.tensor_copy(
                gs1_t[0:64, pp, j * 256:(j + 1) * 256], ps[0:64, 0:256])

        def emit_gather(hp):
            pp = hp % 2
            _, _, poff = head_slices(hp)
            gin = dram.tile([128, N], FP8, tag="gin", name=f"gin{hp}")
            gout = dram.tile([2, 128, N], FP8, tag="gout", bufs=4,
                             name=f"gout{hp}")
            nc.gpsimd.dma_start(gin[0:64, :], gs_t[0:64, pp, :])
            nc.gpsimd.dma_start(gin[64:128, :], gs1_t[0:64, pp, :])
            nc.gpsimd.collective_compute(
                "AllGather", ALU.bypass,
                replica_groups=[[0, 1], [2, 3], [4, 5], [6, 7]],
                ins=[gin.opt()], outs=[gout.opt()])
            for r in range(2):
                tt = 2 * r + hp // 2
                nc.gpsimd.dma_start(cm["2"][poff:poff + 64, tt, :],
                                    gout[r, 0:64, :])
                nc.gpsimd.dma_start(cm["1"][poff:poff + 64, tt, :],
                                    gout[r, 64:128, :])

        def emit_prev_work(hl, qt):
            # head hl-1's epilogue interleaved into head hl's qt loop
            hp = hl - 1
            if qt <= 3:
                emit_ctx2_pairs(hp, 0, (2 * qt, 2 * qt + 1))
            elif qt == 4:
                emit_ctx2_evac(hp, 0)
                emit_ctx2_pairs(hp, 1, (0, 1))
            elif qt in (5, 6):
                emit_ctx2_pairs(hp, 1, (2 * qt - 8, 2 * qt - 7))
            elif qt == 7:
                emit_ctx2_pairs(hp, 1, (6, 7))
            elif qt == 8:
                emit_ctx2_evac(hp, 1)
                emit_cs_col(hp)
            elif qt == 9:
                emit_cs_col2(hp)
            elif 10 <= qt <= 13:
                emit_ctx1_chunk(hp, 2 * (qt - 10))
                emit_ctx1_chunk(hp, 2 * (qt - 10) + 1)
            elif qt == 14:
                emit_gather(hp)

        for hl in range(HL):
            st[hl] = {}
            for qt in range(NT):
                emit_scores(hl, qt)
                emit_exp(hl, qt)
                if hl == 0:
                    emit_v_proj(1, wv2_s, bv2_s, v2tok, qt, False)
                    emit_v_proj(0, wv1_s, bv1_s, v1tok, qt, True)
                else:
                    emit_prev_work(hl, qt)
                emit_transpose(hl, qt)
                if qt % 4 == 3:
                    emit_v2p_group(hl, qt // 4)
            if hl == 0:
                # x tiles retire with head 0's V projections; reuse their
                # SBUF for the residual slices (needed only at P3)
                xb_stack.close()
                xr_pool = ctx.enter_context(tc.tile_pool(name="xr", bufs=1))
                xres = xr_pool.tile([128, 2, 2, N], BF16, tag="xres")
                xres_box[0] = xres
                for m in range(2):
                    nc.gpsimd.dma_start(xres[:, m, 0, :], x1r[m, :, :])
                    nc.gpsimd.dma_start(xres[:, m, 1, :], x2r[m, :, :])

        # last head epilogue
        hp = HL - 1
        for p in range(0, 8, 2):
            emit_ctx2_pairs(hp, 0, (p, p + 1))
        emit_ctx2_evac(hp, 0)
        for p in range(0, 8, 2):
            emit_ctx2_pairs(hp, 1, (p, p + 1))
        emit_ctx2_evac(hp, 1)
        emit_cs_col(hp)
        emit_cs_col2(hp)
        for j in range(8):
            emit_ctx1_chunk(hp, j)
        emit_gather(hp)

        p2.close()

        # ---- P3: output projections (fp8 DoubleRow) + residual ----
        p3 = ExitStack()
        o_ps = p3.enter_context(tc.tile_pool(name="o_ps", bufs=2, space="PSUM"))
        out_pool = p3.enter_context(tc.tile_pool(name="outp", bufs=2))
        xres = xres_box[0]
        for si, (w_s, cmt, oo) in enumerate(((wo2_s, cm["2"], o2),
                                             (wo1_s, cm["1"], o1))):
            stream = 1 - si  # xres index: 0 = x1, 1 = x2
            for m in range(2):
                for cq in range(4):
                    ps = o_ps.tile([128, 512], F32, tag="o")
                    for c2 in range(2):
                        csl = slice((2 * cq + c2) * 256, (2 * cq + c2 + 1) * 256)
                        for pr in range(2):
                            nc.tensor.matmul(
                                ps[:, c2 * 256:(c2 + 1) * 256],
                                w_s[:, 2 * pr:2 * pr + 2, m * 128:(m + 1) * 128],
                                cmt[:, 2 * pr:2 * pr + 2, csl],
                                start=(pr == 0), stop=(pr == 1), perf_mode=DR)
                    ot = out_pool.tile([128, 512], F32, tag="ot")
                    osl = slice(cq * 512, (cq + 1) * 512)
                    nc.vector.scalar_tensor_tensor(
                        ot[:, :], ps[:, :], OUS, xres[:, m, stream, osl],
                        op0=ALU.mult, op1=ALU.add)
                    nc.gpsimd.dma_start(oo[m, :, osl], ot[:, :])
        p3.close()


_NC_CACHE = None


def _get_nc():
    global _NC_CACHE
    if _NC_CACHE is None:
        _NC_CACHE = _build()
    return _NC_CACHE


def _in_maps(x1, x2, Wq, bq, Wk, bk, Wv1, bv1, Wv2, bv2, Wo1, bo1, Wo2, bo2):
    x1f = np.asarray(x1, np.float32).reshape(B, C, N)
    x2f = np.asarray(x2, np.float32).reshape(B, C, N)
    in_maps = []
    for c in range(N_CORES):
        b, hq = c // 2, c % 2
        sl = slice(CL * hq, CL * hq + CL)

        def wslice(W, scale=1.0, dt=_BF):
            return np.ascontiguousarray(
                (np.asarray(W, np.float32)[:, sl] * scale).reshape(CT, 128, CL)
                .transpose(1, 0, 2)).astype(dt)

        m = {
            "x1b": x1f[b].reshape(CT, 128, N).astype(_BF),
            "x2b": x2f[b].reshape(CT, 128, N).astype(_BF),
            "wq": wslice(Wq), "wk": wslice(Wk),
            "wv1": wslice(Wv1), "wv2": wslice(Wv2),
            "wo1": wslice(Wo1, WOS, _F8), "wo2": wslice(Wo2, WOS, _F8),
            "bq": np.ascontiguousarray(
                np.asarray(bq, np.float32)[sl].reshape(2, 128).T).reshape(128, 2, 1),
            "bk": np.ascontiguousarray(
                np.asarray(bk, np.float32)[sl].reshape(2, 128).T).reshape(128, 2, 1),
            "bv1": np.asarray(bv1, np.float32)[sl].reshape(1, CL).astype(_BF),
            "bv2": np.asarray(bv2, np.float32)[sl].reshape(1, CL).astype(_BF),
            "x1r": (x1f[b, sl, :] + np.asarray(bo1, np.float32)[sl, None]
                    ).reshape(2, 128, N).astype(_BF),
            "x2r": (x2f[b, sl, :] + np.asarray(bo2, np.float32)[sl, None]
                    ).reshape(2, 128, N).astype(_BF),
        }
        in_maps.append(m)
    return in_maps


def _unshard(res):
    o1 = np.empty((B, C, N), np.float32)
    o2 = np.empty((B, C, N), np.float32)
    for c in range(N_CORES):
        b, hq = c // 2, c % 2
        sl = slice(CL * hq, CL * hq + CL)
        o1[b, sl, :] = res[c]["o1"].reshape(CL, N)
        o2[b, sl, :] = res[c]["o2"].reshape(CL, N)
    shape = (B, C, 8, 16, 16)
    return o1.reshape(shape), o2.reshape(shape)


def kernel(**inputs):
    in_maps = _in_maps(**inputs)
    nc = _get_nc()
    res = run_bass_kernel_spmd(nc, in_maps, list(range(N_CORES))).results
    return _unshard(res)


# revision 22
# speedup vs baseline: 1.2889x; 1.0379x over previous
"""Trainium2 Bass kernel for the dual-softmax cross-attention module.

Sharding: 8 cores = batch (4) x head-half (2).  Core c handles batch c//2 and
heads 4*(c%2) .. 4*(c%2)+4, producing a disjoint 256-channel slice of both
outputs (context halves exchanged with the pair core via a 2-core AllGather).

Per head: scores run in bf16 (K=64); one big exp ACTIVATE per q-tile writes
E straight to fp8 with the row-sum accumulated on the ACT side.  Both context
matmuls and the output projections run as fp8 DoubleRow (two 128-deep
contraction blocks per instruction).  E^T for ctx1 is produced by DMA-xbar
transposes of the fp8 E viewed as uint16 pairs; the resulting parity
interleave is absorbed by stride-2 access patterns and a pre-permuted
token-major V1.  v1/v2 are pre-scaled by 1024/colsum resp. 1024/rowsum, Wo by
64; the output projection evac fuses the 2^-16 unscale with the residual add.
"""

import sys

for _p in ("/opt/trn_rl_repo", "/opt/pypackages"):
    if _p not in sys.path:
        sys.path.insert(0, _p)

import numpy as np
import ml_dtypes

import concourse.bass as bass
import concourse.tile as tile
from concourse import bacc, mybir
from concourse.bass_utils import run_bass_kernel_spmd

F32 = mybir.dt.float32
BF16 = mybir.dt.bfloat16
FP8 = mybir.dt.float8e4
U16 = mybir.dt.uint16
AF = mybir.ActivationFunctionType
ALU = mybir.AluOpType
DR = mybir.MatmulPerfMode.DoubleRow

N_CORES = 8
B = 4          # batch
C = 512        # channels
N = 2048       # tokens (8*16*16)
H = 8          # heads
DH = 64        # head dim
HL = 4         # heads per core
CL = 256       # channels per core (head-group)
NT = N // 128  # 16 token tiles
CT = C // 128  # 4 channel tiles
ES_RING = 20   # q-tile ring slots for E (fp8)
ET_RING = 28   # ring slots for E^T
VS = 1024.0    # v1/v2 scale (keeps fp8 operands in range)
WOS = 64.0     # Wo scale
OUS = 1.0 / (VS * WOS)  # output unscale

_BF = ml_dtypes.bfloat16
_F8 = ml_dtypes.float8_e4m3


def _build():
    nc = bacc.Bacc("TRN2", target_bir_lowering=False, debug=False,
                   num_devices=N_CORES)

    def din(name, shape, dt=BF16):
        return nc.dram_tensor(name, shape, dt, kind="ExternalInput").ap()

    x1b = din("x1b", [CT, 128, N])          # x1[b] channel-major, bf16
    x2b = din("x2b", [CT, 128, N])
    wq = din("wq", [128, CT, CL])           # column slice of Wq, pre-permuted
    wk = din("wk", [128, CT, CL])
    wv1 = din("wv1", [128, CT, CL])
    wv2 = din("wv2", [128, CT, CL])
    wo1 = din("wo1", [128, CT, CL], FP8)    # Wo columns for my rows, x64 fp8
    wo2 = din("wo2", [128, CT, CL], FP8)
    bq = din("bq", [128, 2, 1], F32)        # bias slices per M-tile
    bk = din("bk", [128, 2, 1], F32)
    bv1 = din("bv1", [1, CL])
    bv2 = din("bv2", [1, CL])
    x1r = din("x1r", [2, 128, N], BF16)     # x1[b] residual slice + bo1
    x2r = din("x2r", [2, 128, N], BF16)

    o1 = nc.dram_tensor("o1", [2, 128, N], F32, kind="ExternalOutput").ap()
    o2 = nc.dram_tensor("o2", [2, 128, N], F32, kind="ExternalOutput").ap()

    with tile.TileContext(nc) as tc:
        _emit(nc, tc, locals())
    nc.compile()
    return nc


def _emit(nc, tc, t):
    x1b, x2b = t["x1b"], t["x2b"]
    wq, wk, wv1, wv2 = t["wq"], t["wk"], t["wv1"], t["wv2"]
    wo1, wo2 = t["wo1"], t["wo2"]
    bq, bk, bv1, bv2 = t["bq"], t["bk"], t["bv1"], t["bv2"]
    x1r, x2r, o1, o2 = t["x1r"], t["x2r"], t["o1"], t["o2"]

    from contextlib import ExitStack
    ctx = ExitStack()
    with ctx:
        persist = ctx.enter_context(tc.tile_pool(name="persist", bufs=1))
        dram = ctx.enter_context(tc.tile_pool(name="dram", bufs=2, space="DRAM"))

        # ---- persistent SBUF tensors ----
        w_all = persist.tile([128, 4, CT, CL], BF16, tag="wall")
        wq_s, wk_s, wv1_s, wv2_s = (w_all[:, i, :, :] for i in range(4))
        wo_all = persist.tile([128, 2, CT, CL], FP8, tag="woall")
        wo1_s, wo2_s = wo_all[:, 0, :, :], wo_all[:, 1, :, :]
        bqk_s = persist.tile([128, 4, 1], F32, tag="bqk")
        bq_s, bk_s = bqk_s[:, 0:2, :], bqk_s[:, 2:4, :]
        misc = persist.tile([128, 640], BF16, tag="misc")
        bv1_s = misc[0:1, 0:CL]
        bv2_s = misc[0:1, CL:2 * CL]
        ones_s = misc[0:1, 512:640]         # ones row for V bias matmul
        qt_s = persist.tile([128, 2, N], BF16, tag="qt")    # Q^T (chan-major)
        kt_s = persist.tile([128, 2, N], BF16, tag="kt")
        v1tok = persist.tile([128, NT, CL], BF16, tag="v1tok")  # PERMUTED ord
        v2tok = persist.tile([128, NT, CL], BF16, tag="v2tok")  # natural ord
        v2p = persist.tile([128, 2, NT, 80], FP8, tag="v2p")    # 64 + 2 ones
        # (pair-dim stride must be a multiple of 16 for dual-fp8 ldweights)
        v1q = persist.tile([128, 2, NT, DH], FP8, tag="v1q")
        rsml = persist.tile([128, 2, 4 * NT], F32, tag="rsml")
        rs_t = rsml[:, :, 0:2 * NT]         # rowsum halves (2 per qt)
        rss_t = rsml[:, :, 2 * NT:3 * NT]   # merged rowsum
        rr_t = rsml[:, :, 3 * NT:4 * NT]    # 1 / rowsum
        csrow = persist.tile([1, N], BF16, tag="csrow")     # colsum row *2^-10
        csc = persist.tile([128, 2 * NT], F32, tag="csc")
        cscol = csc[:, 0:NT]                # colsum col (permuted order)
        crq = csc[:, NT:2 * NT]             # VS / colsum
        cm = {"1": persist.tile([128, CT, N], FP8, tag="cm1", name="cm1"),
              "2": persist.tile([128, CT, N], FP8, tag="cm2", name="cm2")}
        gs_t = persist.tile([128, 2, N], FP8, tag="gs")     # ctx2 staging
        gs1_t = persist.tile([64, 2, N], FP8, tag="gs1")    # ctx1 staging
        xres_box = {}  # [m, stream] residual tile; pool reuses xb's space

        # ---- E / E^T rings (sub-tile AP dependency tracking) ----
        es8 = persist.tile([128, ES_RING, N], FP8, tag="es8")
        esT = persist.tile([128, 8, ET_RING, 256], FP8, tag="esT")
        es8_u16 = es8.bitcast(U16)          # [128, ES_RING, 1024]
        esT_u16 = esT.bitcast(U16)          # [128, 8, ET_RING, 128]

        # ---- input DMA (gpsimd software DGE; sync is reserved for xbar) ----
        for i, src in enumerate((wq, wk, wv1, wv2)):
            nc.gpsimd.dma_start(w_all[:, i, :, :], src[:, :, :])
        nc.gpsimd.dma_start(wo_all[:, 0, :, :], wo1[:, :, :])
        nc.gpsimd.dma_start(wo_all[:, 1, :, :], wo2[:, :, :])
        nc.gpsimd.dma_start(bq_s[:, :, :], bq[:, :, :])
        nc.gpsimd.dma_start(bk_s[:, :, :], bk[:, :, :])
        nc.gpsimd.dma_start(bv1_s[:, :], bv1[:, :])
        nc.gpsimd.dma_start(bv2_s[:, :], bv2[:, :])
        nc.vector.memset(ones_s[:, :], 1.0)
        nc.vector.memset(v2p[:, :, :, DH:DH + 2], 1.0)

        # ---- P1: x loads + Q/K projections ----
        p1 = ExitStack()
        pj_ps = p1.enter_context(tc.tile_pool(name="pj_ps", bufs=2, space="PSUM"))
        xb_stack = ExitStack()
        xb_pool = xb_stack.enter_context(tc.tile_pool(name="xb", bufs=8))
        xts = {}
        for xi, xb_dram in enumerate((x1b, x2b)):
            xts[xi] = [xb_pool.tile([128, N], BF16, tag="xb", name=f"xt{xi}_{i}")
                       for i in range(CT)]
            for ti in range(CT):
                nc.gpsimd.dma_start(xts[xi][ti][:, :], xb_dram[ti, :, :])
        # chan-major Q/K: out[cl, n] = sum_cin W[cin, cl] * x[cin, n] + b
        # m=0 (heads 0-1) first so head 0's scores can start early.
        for m in range(2):
            for xi, w_qk, b_qk, qk_dst in ((0, wq_s, bq_s, qt_s),
                                           (1, wk_s, bk_s, kt_s)):
                for half in range(2):
                    ps = pj_ps.tile([128, 1024], F32, tag="pj")
                    for ch in range(2):
                        off = half * 1024 + ch * 512
                        for ti in range(CT):
                            nc.tensor.matmul(
                                ps[:, ch * 512:(ch + 1) * 512],
                                w_qk[:, ti, m * 128:(m + 1) * 128],
                                xts[xi][ti][:, off:off + 512],
                                start=(ti == 0), stop=(ti == CT - 1))
                    nc.vector.tensor_scalar_add(
                        qk_dst[:, m, half * 1024:(half + 1) * 1024], ps[:, :],
                        b_qk[:, m, :])
        p1.close()

        # ---- head-phase PSUM pools: scores 4 banks, ctx2 2, ctx1 2 ----
        p2 = ExitStack()
        sc_ps = p2.enter_context(tc.tile_pool(name="sc_ps", bufs=2, space="PSUM"))
        c2_ps = p2.enter_context(tc.tile_pool(name="c2_ps", bufs=1, space="PSUM"))
        c1_ps = p2.enter_context(tc.tile_pool(name="c1_ps", bufs=2, space="PSUM"))

        st = {}  # per-head state

        def head_slices(hl):
            g, poff = hl // 2, 64 * (hl % 2)
            return (qt_s[poff:poff + 64, g, :], kt_s[poff:poff + 64, g, :], poff)

        def es_slot(hl, qt):
            return (16 * hl + qt) % ES_RING

        def et_slot(hl, qt):
            return (16 * hl + qt) % ET_RING

        def emit_v_proj(xi, w_v, b_v, v_dst, sl, permute):
            # token-major V: out[n, cl] = sum_cin x[cin, n] * W[cin, cl] + bv
            # permute: stationary picks tokens 256*(sl//2) + (sl%2) :: 2
            ps = c1_ps.tile([128, 512], F32, tag="c1", name=f"vps{xi}_{sl}")
            for ti in range(CT):
                xt = xts[xi][ti]
                if permute:
                    a0 = 128 * (sl // 2)
                    j = sl % 2
                    src = (xt[:, :].rearrange("p (a s) -> p a s", s=2)
                           [:, a0:a0 + 128, j:j + 1])
                else:
                    src = xt[:, sl * 128:(sl + 1) * 128]
                nc.tensor.matmul(ps[:, 0:CL], src, w_v[:, ti, :],
                                 start=(ti == 0), stop=False)
            nc.tensor.matmul(ps[:, 0:CL], ones_s[:, 0:128], b_v[:, :],
                             start=False, stop=True)
            nc.vector.tensor_scalar_mul(v_dst[:, sl, :], ps[:, 0:CL], VS)

        def emit_scores_half(hl, qt, half):
            q_l, k_l, _ = head_slices(hl)
            ps = sc_ps.tile([128, 1024], F32, tag="sc", name=f"sc{hl}_{qt}_{half}")
            st[hl]["sc%d" % half] = ps
            for u in range(2):
                ku = half * 2 + u
                nc.tensor.matmul(ps[:, u * 512:(u + 1) * 512],
                                 q_l[:, qt * 128:(qt + 1) * 128],
                                 k_l[:, ku * 512:(ku + 1) * 512],
                                 start=True, stop=True)

        def emit_exp_half(hl, qt, half):
            s = es_slot(hl, qt)
            pp = hl % 2
            nc.scalar.activation(es8[:, s, half * 1024:(half + 1) * 1024],
                                 st[hl]["sc%d" % half][:, :], AF.Exp,
                                 scale=0.125,
                                 accum_out=rs_t[:, pp, 2 * qt + half:2 * qt + half + 1])

        def emit_transpose(hl, qt):
            s, ts = es_slot(hl, qt), et_slot(hl, qt)
            nc.sync.dma_start(esT_u16[:, :, ts, :], es8_u16[:, s, :],
                              transpose=True)

        def emit_v2p_group(hl, g):
            # rowsum = halfA + halfB; rr = 1/rowsum; v2tok already holds VS*v2
            pp = hl % 2
            sl4 = slice(4 * g, 4 * g + 4)
            rsh = rs_t[:, pp, :].rearrange("p (q h) -> p h q", h=2)
            nc.vector.tensor_add(rss_t[:, pp, sl4],
                                 rsh[:, 0, 4 * g:4 * g + 4],
                                 rsh[:, 1, 4 * g:4 * g + 4])
            nc.vector.reciprocal(rr_t[:, pp, sl4], rss_t[:, pp, sl4])
            for qt in range(4 * g, 4 * g + 4):
                nc.vector.tensor_scalar_mul(
                    v2p[:, pp, qt, 0:DH], v2tok[:, qt, hl * DH:(hl + 1) * DH],
                    rr_t[:, pp, qt:qt + 1])

        def emit_ctx2_pairs(hp, half, pairs):
            # ctx2 chan-major [65, 1024-half], fp8 DoubleRow over qt pairs
            pp = hp % 2
            s = st[hp]
            for p in pairs:
                if p == 0:
                    s["c2"] = c2_ps.tile([128, 1024], F32, tag="c2",
                                         name=f"c2_{hp}_{half}")
                sl0 = es_slot(hp, 2 * p)
                for ch in range(2):
                    ksl = slice(half * 1024 + ch * 512,
                                half * 1024 + (ch + 1) * 512)
                    nc.tensor.matmul(
                        s["c2"][0:DH + 2, ch * 512:(ch + 1) * 512],
                        v2p[:, pp, 2 * p:2 * p + 2, 0:DH + 2],
                        es8[:, sl0:sl0 + 2, ksl],
                        start=(p == 0), stop=(p == 7), perf_mode=DR)

        def emit_ctx2_evac(hp, half):
            pp = hp % 2
            s = st[hp]
            nc.vector.tensor_copy(gs_t[0:DH, pp, half * 1024:(half + 1) * 1024],
                                  s["c2"][0:DH, :])
            # colsum row, written directly in the permuted (u, s=2*ub+j)
            # order v1tok uses: csrow[u*16 + 2*ub + j] = colsum[256*ub+2u+j]
            dst = (csrow[0:1, :]
                   .rearrange("o (u ub j) -> o ub u j", u=128, ub=8, j=2)
                   [:, 4 * half:4 * half + 4, :, :])
            nc.vector.tensor_copy(dst, s["c2"][DH:DH + 1, :])

        def emit_cs_col(hp):
            # csrow is already in (u, s) order; spread over 128 partitions
            nc.gpsimd.dma_start(cscol[:, :], csrow[0:1, :])

        def emit_cs_col2(hp):
            nc.vector.reciprocal(crq[:, :], cscol[:, :])
            pp = hp % 2
            for sl in range(NT):
                nc.vector.tensor_scalar_mul(
                    v1q[:, pp, sl, :], v1tok[:, sl, hp * DH:(hp + 1) * DH],
                    crq[:, sl:sl + 1])

        def emit_ctx1_chunk(hp, j):
            # ctx1 chan-major [64 d, 512 q]: lhsT = v1q pair (stationary),
            # rhs = E^T pairs with the parity interleave as the DoubleRow dim
            pp = hp % 2
            ps = c1_ps.tile([64, 512], F32, tag="c1", name=f"c1_{hp}_{j}")
            ts0 = et_slot(hp, 4 * j)
            rhs = (esT[:, :, ts0:ts0 + 4, :]
                   .rearrange("p ub t (q j) -> p ub j t q", j=2))
            for ub in range(8):
                nc.tensor.matmul(
                    ps[0:64, 0:512],
                    v1q[:, pp, 2 * ub:2 * ub + 2, :],
                    rhs[:, ub, :, :, :],
                    start=(ub == 0), stop=(ub == 7), perf_mode=DR)
            nc.vector.tensor_copy(
                gs1_t[0:64, pp, j * 512:(j + 1) * 512], ps[0:64, 0:512])

        def emit_gather(hp):
            pp = hp % 2
            _, _, poff = head_slices(hp)
            gin = dram.tile([128, N], FP8, tag="gin", name=f"gin{hp}")
            gout = dram.tile([2, 128, N], FP8, tag="gout", bufs=4,
                             name=f"gout{hp}")
            nc.gpsimd.dma_start(gin[0:64, :], gs_t[0:64, pp, :])
            nc.gpsimd.dma_start(gin[64:128, :], gs1_t[0:64, pp, :])
            nc.gpsimd.collective_compute(
                "AllGather", ALU.bypass,
                replica_groups=[[0, 1], [2, 3], [4, 5], [6, 7]],
                ins=[gin.opt()], outs=[gout.opt()])
            for r in range(2):
                tt = 2 * r + hp // 2
                nc.gpsimd.dma_start(cm["2"][poff:poff + 64, tt, :],
                                    gout[r, 0:64, :])
                nc.gpsimd.dma_start(cm["1"][poff:poff + 64, tt, :],
                                    gout[r, 64:128, :])

        def emit_prev_work(hl, qt):
            # head hl-1's epilogue interleaved into head hl's qt loop
            hp = hl - 1
            if qt <= 3:
                emit_ctx2_pairs(hp, 0, (2 * qt, 2 * qt + 1))
            elif qt == 4:
                emit_ctx2_evac(hp, 0)
                emit_ctx2_pairs(hp, 1, (0, 1))
            elif qt in (5, 6):
                emit_ctx2_pairs(hp, 1, (2 * qt - 8, 2 * qt - 7))
            elif qt == 7:
                emit_ctx2_pairs(hp, 1, (6, 7))
            elif qt == 8:
                emit_ctx2_evac(hp, 1)
                emit_cs_col(hp)
            elif qt == 9:
                emit_cs_col2(hp)
            elif 10 <= qt <= 13:
                emit_ctx1_chunk(hp, qt - 10)
            elif qt == 14:
                emit_gather(hp)

        for hl in range(HL):
            st[hl] = {}
            for qt in range(NT):
                emit_scores_half(hl, qt, 0)
                emit_exp_half(hl, qt, 0)
                emit_scores_half(hl, qt, 1)
                emit_exp_half(hl, qt, 1)
                if hl == 0:
                    emit_v_proj(1, wv2_s, bv2_s, v2tok, qt, False)
                    emit_v_proj(0, wv1_s, bv1_s, v1tok, qt, True)
                else:
                    emit_prev_work(hl, qt)
                emit_transpose(hl, qt)
                if qt % 4 == 3:
                    emit_v2p_group(hl, qt // 4)
            if hl == 0:
                # x tiles retire with head 0's V projections; reuse their
                # SBUF for the residual slices (needed only at P3)
                xb_stack.close()
                xr_pool = ctx.enter_context(tc.tile_pool(name="xr", bufs=1))
                xres = xr_pool.tile([128, 2, 2, N], BF16, tag="xres")
                xres_box[0] = xres
                for m in range(2):
                    nc.gpsimd.dma_start(xres[:, m, 0, :], x1r[m, :, :])
                    nc.gpsimd.dma_start(xres[:, m, 1, :], x2r[m, :, :])

        # last head epilogue
        hp = HL - 1
        for p in range(0, 8, 2):
            emit_ctx2_pairs(hp, 0, (p, p + 1))
        emit_ctx2_evac(hp, 0)
        for p in range(0, 8, 2):
            emit_ctx2_pairs(hp, 1, (p, p + 1))
        emit_ctx2_evac(hp, 1)
        emit_cs_col(hp)
        emit_cs_col2(hp)
        for j in range(4):
            emit_ctx1_chunk(hp, j)
        emit_gather(hp)

        p2.close()

        # ---- P3: output projections (fp8 DoubleRow) + residual ----
        p3 = ExitStack()
        o_ps = p3.enter_context(tc.tile_pool(name="o_ps", bufs=2, space="PSUM"))
        out_pool = p3.enter_context(tc.tile_pool(name="outp", bufs=2))
        xres = xres_box[0]
        for si, (w_s, cmt, oo) in enumerate(((wo2_s, cm["2"], o2),
                                             (wo1_s, cm["1"], o1))):
            stream = 1 - si  # xres index: 0 = x1, 1 = x2
            for m in range(2):
                for cq in range(4):
                    ps = o_ps.tile([128, 512], F32, tag="o")
                    osl = slice(cq * 512, (cq + 1) * 512)
                    for pr in range(2):
                        nc.tensor.matmul(
                            ps[:, :],
                            w_s[:, 2 * pr:2 * pr + 2, m * 128:(m + 1) * 128],
                            cmt[:, 2 * pr:2 * pr + 2, osl],
                            start=(pr == 0), stop=(pr == 1), perf_mode=DR)
                    ot = out_pool.tile([128, 512], F32, tag="ot")
                    nc.vector.scalar_tensor_tensor(
                        ot[:, :], ps[:, :], OUS, xres[:, m, stream, osl],
                        op0=ALU.mult, op1=ALU.add)
                    nc.gpsimd.dma_start(oo[m, :, osl], ot[:, :])
        p3.close()


_NC_CACHE = None


def _get_nc():
    global _NC_CACHE
    if _NC_CACHE is None:
        _NC_CACHE = _build()
    return _NC_CACHE


def _in_maps(x1, x2, Wq, bq, Wk, bk, Wv1, bv1, Wv2, bv2, Wo1, bo1, Wo2, bo2):
    x1f = np.asarray(x1, np.float32).reshape(B, C, N)
    x2f = np.asarray(x2, np.float32).reshape(B, C, N)
    in_maps = []
    for c in range(N_CORES):
        b, hq = c // 2, c % 2
        sl = slice(CL * hq, CL * hq + CL)

        def wslice(W, scale=1.0, dt=_BF):
            return np.ascontiguousarray(
                (np.asarray(W, np.float32)[:, sl] * scale).reshape(CT, 128, CL)
                .transpose(1, 0, 2)).astype(dt)

        m = {
            "x1b": x1f[b].reshape(CT, 128, N).astype(_BF),
            "x2b": x2f[b].reshape(CT, 128, N).astype(_BF),
            "wq": wslice(Wq), "wk": wslice(Wk),
            "wv1": wslice(Wv1), "wv2": wslice(Wv2),
            "wo1": wslice(Wo1, WOS, _F8), "wo2": wslice(Wo2, WOS, _F8),
            "bq": np.ascontiguousarray(
                np.asarray(bq, np.float32)[sl].reshape(2, 128).T).reshape(128, 2, 1),
            "bk": np.ascontiguousarray(
                np.asarray(bk, np.float32)[sl].reshape(2, 128).T).reshape(128, 2, 1),
            "bv1": np.asarray(bv1, np.float32)[sl].reshape(1, CL).astype(_BF),
            "bv2": np.asarray(bv2, np.float32)[sl].reshape(1, CL).astype(_BF),
            "x1r": (x1f[b, sl, :] + np.asarray(bo1, np.float32)[sl, None]
                    ).reshape(2, 128, N).astype(_BF),
            "x2r": (x2f[b, sl, :] + np.asarray(bo2, np.float32)[sl, None]
                    ).reshape(2, 128, N).astype(_BF),
        }
        in_maps.append(m)
    return in_maps


def _unshard(res):
    o1 = np.empty((B, C, N), np.float32)
    o2 = np.empty((B, C, N), np.float32)
    for c in range(N_CORES):
        b, hq = c // 2, c % 2
        sl = slice(CL * hq, CL * hq + CL)
        o1[b, sl, :] = res[c]["o1"].reshape(CL, N)
        o2[b, sl, :] = res[c]["o2"].reshape(CL, N)
    shape = (B, C, 8, 16, 16)
    return o1.reshape(shape), o2.reshape(shape)


def kernel(**inputs):
    in_maps = _in_maps(**inputs)
    nc = _get_nc()
    res = run_bass_kernel_spmd(nc, in_maps, list(range(N_CORES))).results
    return _unshard(res)
